# revision 41
# baseline (speedup 1.0000x reference)
"""Multi-head self-attention (B=4, N=2048, C=512, H=8) on 8 trn2 NeuronCores.

Sharding: core = 2*b + g (b = batch, g = head-half). Each core handles one
batch element and 4 heads (2 head-pairs j); host sums the two partial
projections per batch element and adds b_proj.

v4 design (all attention matmuls bf16, fp32 PSUM accumulation):
  0. startup: inputs spread over the three dma-capable queues (sync/
     scalar/gpsimd, ~90 B/ns each; a dma_start kick costs ~0.7us of
     engine time, so kick order = critical path order); 24 dummy
     accumulating matmuls on memset data warm the PE clock (HAM 1.2->2.4
     GHz) while x streams in.
  1. qkv: q^T/k^T per pair j as [128, 2048] bf16, ct-outer in nkq-pairs
     so the first matmuls need only the first x tiles. v packed per
     key-tile as [128, 4, 66] bf16 with a ones column at index 64
     (softmax-denominator trick); v matmuls share the qk PSUM pool.
     Early-exp: scores+exp for (j0,q5=0) run during the qkv phase
     interleaved with the v matmuls, alternating ACT/DVE per key-tile.
  2. attention, software-pipelined: scores for key-tile m+LOOK issue
     ahead of AV matmuls for tile m, so exp (ACT, with DVE-Schraudolph
     tiles per DVE_MS/SPLIT_M balancing the two queues) is never on the
     PE's critical path. PSUM: 3 score tiles (6 banks) + oTa/oTb (2).
     Group-1 scores pre-issue through group-0's early AVs (PRE).
  3. normalize, split: prompt part frees the PSUM banks (den-row copy on
     DVE, body copy on ACT); reciprocal + Pool broadcast + DVE multiply
     are deferred into the next group's stream so they never block exps.
     NB: custom DVE ops (reciprocal) silently corrupt on non-zero base
     partitions on HW (sim does not model this), and ACT APs must be
     32-aligned in partitions - hence the staging copies.
  4. projection: y^T accumulated over the two pairs per q-chunk, bf16
     output (host gathers in fp32), 16 output DMAs spread over the three
     queues so the 2MB drain overlaps the last attention group.
"""

from collections import deque

import numpy as np

import concourse.bacc as bacc
import concourse.bass as bass
import concourse.mybir as mybir
import concourse.tile as tile
from concourse.bass_utils import run_bass_kernel_spmd

B, N, C, H, HD = 4, 2048, 512, 8, 64
HPC, CS = 4, 256  # heads per core, channels per core
SCALE = HD ** -0.5
F32 = mybir.dt.float32
BF16 = mybir.dt.bfloat16
U16 = mybir.dt.uint16
NCORES = 8
MT = N // 128  # 16 key tiles

LOG2E = float(np.log2(np.e))
SCH_A = SCALE * 128.0 * LOG2E   # schraudolph scale (bf16 bits)
SCH_B = 16256.0 - 5.5           # 127<<7 minus minimax correction

# key-tiles m whose exp runs on DVE (Schraudolph); rest use exact ACT exp
DVE_MS = frozenset({1, 3, 5, 7, 9, 13})
SPLIT_M = 15  # this tile's exp is half ACT / half DVE
EARLY_EXP = True
LOOK = 5  # score tiles issued ahead of AV matmuls

_NC = None


def _build(reps=1):
    nc = bacc.Bacc("TRN2", target_bir_lowering=False, debug=False,
                   num_devices=NCORES)
    xT_d = nc.dram_tensor("xT", [C, N], BF16, kind="ExternalInput")
    wqT_d = nc.dram_tensor("wqT", [C, CS], BF16, kind="ExternalInput")
    wkT_d = nc.dram_tensor("wkT", [C, CS], BF16, kind="ExternalInput")
    wvT_d = nc.dram_tensor("wvT", [C, CS], BF16, kind="ExternalInput")
    wpT_d = nc.dram_tensor("wpT", [CS, C], BF16, kind="ExternalInput")
    bq_d = nc.dram_tensor("bq", [128, 2], F32, kind="ExternalInput")
    bk_d = nc.dram_tensor("bk", [128, 2], F32, kind="ExternalInput")
    bv_d = nc.dram_tensor("bv", [1, CS], BF16, kind="ExternalInput")
    ones4_d = nc.dram_tensor("ones4", [128, HPC], BF16, kind="ExternalInput")
    ones_row_d = nc.dram_tensor("ones_row", [1, 128], BF16,
                                kind="ExternalInput")
    yT_d = nc.dram_tensor("yT", [C, N], BF16, kind="ExternalOutput")

    with tile.TileContext(nc) as tc:
      def body():
          with (
              tc.tile_pool(name="const", bufs=1) as const,
              tc.tile_pool(name="big", bufs=1) as big,
              tc.tile_pool(name="pexp", bufs=8) as pexp,
              tc.tile_pool(name="psch", bufs=8) as psch,
              tc.tile_pool(name="pearly", bufs=1) as pearly,
              tc.tile_pool(name="rc", bufs=3) as rcp,
              tc.tile_pool(name="rbc", bufs=3) as rbcp,
              tc.tile_pool(name="osb", bufs=4) as osbp,
              tc.tile_pool(name="ysb", bufs=4) as ysbp,
          ):
              # ---- input DMA ------------------------------------------------
              xt = [big.tile([128, N], BF16, tag=f"x{ct}", name=f"x{ct}")
                    for ct in range(4)]
              wq_t, wk_t, wv_t = [], [], []
              for ct in range(4):
                  for lst, nm in ((wq_t, "wq"), (wk_t, "wk"), (wv_t, "wv")):
                      lst.append(const.tile([128, CS], BF16, tag=f"{nm}{ct}",
                                            name=f"{nm}{ct}"))
              # critical-path order: qk ct-outer needs xt[0]+wk0/wq0 first;
              # spread x over the three dma-capable queues
              nc.sync.dma_start(out=xt[0][:], in_=xT_d[bass.ts(0, 128), :])
              nc.scalar.dma_start(out=xt[1][:], in_=xT_d[bass.ts(1, 128), :])
              nc.gpsimd.dma_start(out=xt[2][:], in_=xT_d[bass.ts(2, 128), :])
              nc.sync.dma_start(out=xt[3][:, 0:1024],
                                in_=xT_d[bass.ts(3, 128), 0:1024])
              nc.scalar.dma_start(out=xt[3][:, 1024:2048],
                                  in_=xT_d[bass.ts(3, 128), 1024:2048])
              nc.gpsimd.dma_start(out=wk_t[0][:],
                                  in_=wkT_d[bass.ts(0, 128), :])
              nc.gpsimd.dma_start(out=wq_t[0][:],
                                  in_=wqT_d[bass.ts(0, 128), :])
              bk_sb = const.tile([128, 2], F32, tag="bk", name="bk")
              nc.gpsimd.dma_start(out=bk_sb[:], in_=bk_d[:])
              bq_sb = const.tile([128, 2], F32, tag="bq", name="bq")
              nc.gpsimd.dma_start(out=bq_sb[:], in_=bq_d[:])
              nc.sync.dma_start(out=wk_t[1][:], in_=wkT_d[bass.ts(1, 128), :])
              nc.sync.dma_start(out=wk_t[3][:], in_=wkT_d[bass.ts(3, 128), :])
              nc.scalar.dma_start(out=wq_t[1][:],
                                  in_=wqT_d[bass.ts(1, 128), :])
              nc.scalar.dma_start(out=wq_t[3][:],
                                  in_=wqT_d[bass.ts(3, 128), :])
              nc.gpsimd.dma_start(out=wk_t[2][:],
                                  in_=wkT_d[bass.ts(2, 128), :])
              nc.gpsimd.dma_start(out=wq_t[2][:],
                                  in_=wqT_d[bass.ts(2, 128), :])
              for ct in range(4):
                  nc.scalar.dma_start(out=wv_t[ct][:],
                                      in_=wvT_d[bass.ts(ct, 128), :])
              bv_sb = const.tile([1, CS], BF16, tag="bv", name="bv")
              nc.gpsimd.dma_start(out=bv_sb[:], in_=bv_d[:])
              ones_row = const.tile([1, 128], BF16, tag="ones_row",
                                    name="ones_row")
              nc.gpsimd.dma_start(out=ones_row[:], in_=ones_row_d[:])
              wp_t = []
              for j in range(2):
                  t = const.tile([128, C], BF16, tag=f"wp{j}", name=f"wp{j}")
                  nc.gpsimd.dma_start(out=t[:], in_=wpT_d[bass.ts(j, 128), :])
                  wp_t.append(t)

              # ---- persistent activations -----------------------------------
              qT = [big.tile([128, N], BF16, tag=f"qT{j}", name=f"qT{j}")
                    for j in range(2)]
              kT = [big.tile([128, N], BF16, tag=f"kT{j}", name=f"kT{j}")
                    for j in range(2)]
              v1m = [big.tile([128, HPC, HD + 2], BF16, tag=f"v1m_{m}",
                              name=f"v1m_{m}") for m in range(MT)]
              for m in range(MT):
                  nc.sync.dma_start(
                      out=v1m[m][:, :, HD:HD + 1],
                      in_=ones4_d[:, :].rearrange("p (h o) -> p h o", o=1),
                  )
              oT_sb = [big.tile([128, N], BF16, tag=f"oT{j}", name=f"oT{j}")
                       for j in range(2)]

              # ---- phase 1: qkv ---------------------------------------------
              pt_early = []
              with (
                  tc.tile_pool(name="qkps", bufs=2, space="PSUM") as qkps,
                  tc.tile_pool(name="eps", bufs=3, space="PSUM") as eps,
              ):
                  def qk_pair(j):
                      for w_t, b_sb, dst in ((wk_t, bk_sb, kT),
                                             (wq_t, bq_sb, qT)):
                          for nk0 in (0, 2):
                              pss = [qkps.tile([128, 512], F32, tag="qk",
                                               name="qk") for _ in range(2)]
                              for ct in range(4):
                                  for i, ps in enumerate(pss):
                                      nc.tensor.matmul(
                                          ps[:],
                                          lhsT=w_t[ct][:, bass.ts(j, 128)],
                                          rhs=xt[ct][:, bass.ts(nk0 + i,
                                                                512)],
                                          start=(ct == 0), stop=(ct == 3),
                                      )
                              for i, ps in enumerate(pss):
                                  nc.vector.tensor_scalar_add(
                                      dst[j][:, bass.ts(nk0 + i, 512)],
                                      ps[:], b_sb[:, j:j + 1])

                  # dummy matmuls on memset data warm the PE clock (HAM)
                  # during the input DMA so qk runs at 2.4 GHz
                  warm = const.tile([128, 512], BF16, tag="warm",
                                    name="warm")
                  nc.vector.memset(warm[:], 0.0)
                  wps = eps.tile([128, 1024], F32, tag="es", name="warmps")
                  NWARM = 24
                  for i in range(NWARM):
                      nc.tensor.matmul(wps[:, 0:512], lhsT=warm[:, 0:128],
                                       rhs=warm[:], start=(i == 0),
                                       stop=(i == NWARM - 1))
                  qk_pair(0)
                  # early-exp for (j0, q5=0) interleaved with the v matmuls
                  for m in range(MT):
                      if EARLY_EXP:
                          esT = eps.tile([128, 1024], F32, tag="es",
                                         name="es")
                          nc.tensor.matmul(
                              esT[:, 0:512],
                              lhsT=kT[0][0:64, bass.ts(m, 128)],
                              rhs=qT[0][0:64, 0:512],
                              start=True, stop=True)
                          nc.tensor.matmul(
                              esT[:, 512:1024],
                              lhsT=kT[0][64:128, bass.ts(m, 128)],
                              rhs=qT[0][64:128, 0:512],
                              start=True, stop=True)
                          if m % 2 == 0:
                              ept = pearly.tile([128, 1024], BF16,
                                                tag=f"ep{m}", name=f"ep{m}")
                              nc.scalar.activation(
                                  out=ept[:], in_=esT[:],
                                  func=mybir.ActivationFunctionType.Exp,
                                  scale=SCALE)
                              pt_early.append(
                                  (ept[:, 0:512], ept[:, 512:1024]))
                          else:
                              epu = pearly.tile([128, 1024], U16,
                                                tag=f"ep{m}", name=f"ep{m}")
                              nc.vector.tensor_scalar(
                                  out=epu[:], in0=esT[:],
                                  scalar1=SCH_A, scalar2=SCH_B,
                                  op0=mybir.AluOpType.mult,
                                  op1=mybir.AluOpType.add)
                              pt_early.append(
                                  (epu[:, 0:512].bitcast(BF16),
                                   epu[:, 512:1024].bitcast(BF16)))
                      vp = qkps.tile([128, 512], F32, tag="qk", name="vps")
                      for ct in range(4):
                          nc.tensor.matmul(
                              vp[:, 0:CS],
                              lhsT=xt[ct][:, bass.ts(m, 128)],
                              rhs=wv_t[ct][:],
                              start=(ct == 0), stop=False,
                          )
                      nc.tensor.matmul(vp[:, 0:CS], lhsT=ones_row[:],
                                       rhs=bv_sb[:],
                                       start=False, stop=True)
                      if m % 2 == 0:
                          nc.scalar.copy(v1m[m][:, :, 0:HD], vp[:, 0:CS])
                      else:
                          nc.vector.tensor_copy(v1m[m][:, :, 0:HD],
                                                vp[:, 0:CS])
                  qk_pair(1)

              # ---- phase 2: attention (software pipeline) -------------------
              groups = [(j, q5) for j in range(2) for q5 in range(4)]
              with (
                  tc.tile_pool(name="stps", bufs=3, space="PSUM") as stps,
                  tc.tile_pool(name="otps", bufs=1, space="PSUM") as otps,
              ):
                  def issue_scores(gi, m):
                      j, q5 = groups[gi]
                      if EARLY_EXP and gi == 0:
                          return pt_early[m]
                      sT = stps.tile([128, 1024], F32, tag="s", name="s")
                      nc.tensor.matmul(
                          sT[:, 0:512],
                          lhsT=kT[j][0:64, bass.ts(m, 128)],
                          rhs=qT[j][0:64, bass.ts(q5, 512)],
                          start=True, stop=True)
                      nc.tensor.matmul(
                          sT[:, 512:1024],
                          lhsT=kT[j][64:128, bass.ts(m, 128)],
                          rhs=qT[j][64:128, bass.ts(q5, 512)],
                          start=True, stop=True)
                      if m == SPLIT_M:
                          # half-split tile to fine-balance ACT/DVE load
                          pa = pexp.tile([128, 512], BF16, tag="pes",
                                         name="pes")
                          nc.scalar.activation(
                              out=pa[:], in_=sT[:, 0:512],
                              func=mybir.ActivationFunctionType.Exp,
                              scale=SCALE)
                          pb = psch.tile([128, 512], U16, tag="pss",
                                         name="pss")
                          nc.vector.tensor_scalar(
                              out=pb[:], in0=sT[:, 512:1024],
                              scalar1=SCH_A, scalar2=SCH_B,
                              op0=mybir.AluOpType.mult,
                              op1=mybir.AluOpType.add)
                          return (pa[:], pb[:].bitcast(BF16))
                      if m in DVE_MS:
                          pt = psch.tile([128, 1024], U16, tag="ps",
                                         name="ps")
                          nc.vector.tensor_scalar(
                              out=pt[:], in0=sT[:],
                              scalar1=SCH_A, scalar2=SCH_B,
                              op0=mybir.AluOpType.mult,
                              op1=mybir.AluOpType.add)
                          return (pt[:, 0:512].bitcast(BF16),
                                  pt[:, 512:1024].bitcast(BF16))
                      pt = pexp.tile([128, 1024], BF16, tag="pe", name="pe")
                      nc.scalar.activation(
                          out=pt[:], in_=sT[:],
                          func=mybir.ActivationFunctionType.Exp,
                          scale=SCALE)
                      return (pt[:, 0:512], pt[:, 512:1024])

                  def issue_av(gi, m, rhs_ab, oT_pair):
                      j, q5 = groups[gi]
                      for hh, (oT, rhs) in enumerate(zip(oT_pair, rhs_ab)):
                          nc.tensor.matmul(
                              oT[:],
                              lhsT=v1m[m][:, 2 * j + hh, 0:HD + 1],
                              rhs=rhs,
                              start=(m == 0), stop=(m == MT - 1))

                  deferred = []

                  def finalize_a(gi, oT_pair):
                      # prompt part: free the PSUM banks; denominator sits
                      # at partition 0 so recip reads PSUM directly
                      j, q5 = groups[gi]
                      for hh, oT in enumerate(oT_pair):
                          den = rcp.tile([1, 512], F32, tag="den",
                                         name="den")
                          nc.vector.tensor_copy(den[:], oT[HD:HD + 1, :])
                          osb = osbp.tile([HD, 512], F32, tag="osb",
                                          name="osb")
                          nc.scalar.copy(osb[:], oT[0:HD, :])
                          deferred.append((j, q5, hh, den, osb))

                  def finalize_b():
                      while deferred:
                          j, q5, hh, den, osb = deferred.pop(0)
                          rc = rcp.tile([1, 512], F32, tag="rc", name="rc")
                          nc.vector.reciprocal_approx_fast(
                              out=rc[:], in_=den[:])
                          bc = rbcp.tile([HD, 512], F32, tag="bc", name="bc")
                          nc.gpsimd.partition_broadcast(bc[:], rc[:])
                          nc.vector.tensor_mul(
                              oT_sb[j][bass.ts(hh, 64), bass.ts(q5, 512)],
                              osb[:], bc[:])

                  def alloc_pair():
                      return (
                          otps.tile([HD + 1, 512], F32, tag="oa", name="oa"),
                          otps.tile([HD + 1, 512], F32, tag="ob", name="ob"),
                      )

                  if EARLY_EXP:
                      # group 0's AVs read precomputed SBUF tiles, so
                      # group 1's scores+exps interleave through them.
                      # PRE bounded by exp-output capacity (6 ACT + 6 DVE).
                      PRE = 12
                      oT0 = alloc_pair()
                      q1 = deque()
                      for m in range(MT):
                          if 0 <= m - 2 < PRE:
                              q1.append((m - 2, issue_scores(1, m - 2)))
                          issue_av(0, m, pt_early[m], oT0)
                      finalize_a(0, oT0)
                      oT1 = alloc_pair()
                      for m in range(PRE, MT):
                          q1.append((m, issue_scores(1, m)))
                          pm, rhs = q1.popleft()
                          issue_av(1, pm, rhs, oT1)
                          if pm == 4:
                              finalize_b()
                      while q1:
                          pm, rhs = q1.popleft()
                          issue_av(1, pm, rhs, oT1)
                      finalize_a(1, oT1)
                      start_gi = 2
                  else:
                      start_gi = 0

                  for gi in range(start_gi, len(groups)):
                      oT_pair = alloc_pair()
                      q = deque()
                      for m in range(MT):
                          q.append((m, issue_scores(gi, m)))
                          if len(q) > LOOK:
                              pm, rhs = q.popleft()
                              issue_av(gi, pm, rhs, oT_pair)
                              if pm == 4:
                                  finalize_b()
                      while q:
                          pm, rhs = q.popleft()
                          issue_av(gi, pm, rhs, oT_pair)
                      finalize_a(gi, oT_pair)
                  finalize_b()
              # ---- phase 3: projection --------------------------------------
              with tc.tile_pool(name="yps", bufs=4, space="PSUM") as yps:
                  yqs = [nc.sync, nc.gpsimd, nc.scalar]
                  for tch in range(4):
                      for jj in range(4):
                          yp = yps.tile([128, 512], F32, tag="yp",
                                        name="yp")
                          for j in range(2):
                              nc.tensor.matmul(
                                  yp[:],
                                  lhsT=wp_t[j][:, bass.ts(jj, 128)],
                                  rhs=oT_sb[j][:, bass.ts(tch, 512)],
                                  start=(j == 0), stop=(j == 1))
                          ys = ysbp.tile([128, 512], BF16, tag="ys",
                                         name="ys")
                          if jj % 2 == 0:
                              nc.scalar.copy(ys[:], yp[:])
                          else:
                              nc.vector.tensor_copy(ys[:], yp[:])
                          yqs[(4 * tch + jj) % 3].dma_start(
                              out=yT_d[bass.ts(jj, 128), bass.ts(tch, 512)],
                              in_=ys[:])

      if reps > 1:
          with tc.For_i(0, reps, 1):
              body()
      else:
          body()

    nc.compile()
    return nc


def get_nc():
    global _NC
    if _NC is None:
        _NC = _build()
    return _NC


def build_timing_nc(reps):
    return _build(reps=reps)


def shard_inputs(x, w_qkv, b_qkv, w_proj, b_proj):
    import ml_dtypes

    bf16 = ml_dtypes.bfloat16
    x = np.asarray(x, dtype=np.float32)
    w_qkv = np.asarray(w_qkv, dtype=np.float32)
    b_qkv = np.asarray(b_qkv, dtype=np.float32)
    w_proj = np.asarray(w_proj, dtype=np.float32)
    ones4 = np.ones((128, HPC), bf16)
    ones_row = np.ones((1, 128), bf16)
    in_maps = []
    for core in range(NCORES):
        b, g = core // 2, core % 2
        sl = slice(g * CS, (g + 1) * CS)
        in_maps.append({
            "xT": np.ascontiguousarray(x[b].T).astype(bf16),
            "wqT": np.ascontiguousarray(w_qkv[sl, :].T).astype(bf16),
            "wkT": np.ascontiguousarray(w_qkv[C:][sl, :].T).astype(bf16),
            "wvT": np.ascontiguousarray(w_qkv[2 * C:][sl, :].T).astype(bf16),
            "wpT": np.ascontiguousarray(w_proj[:, sl].T).astype(bf16),
            "bq": np.ascontiguousarray(b_qkv[sl].reshape(2, 128).T),
            "bk": np.ascontiguousarray(b_qkv[C:][sl].reshape(2, 128).T),
            "bv": np.ascontiguousarray(
                b_qkv[2 * C:][sl].reshape(1, CS)).astype(bf16),
            "ones4": ones4,
            "ones_row": ones_row,
        })
    return in_maps


def gather_output(results, b_proj):
    b_proj = np.asarray(b_proj, dtype=np.float32)
    out = np.empty((B, N, C), np.float32)
    for b in range(B):
        yT = (results[2 * b]["yT"].astype(np.float32)
              + results[2 * b + 1]["yT"].astype(np.float32))
        out[b] = yT.T + b_proj[None, :]
    return out


def kernel(x, w_qkv, b_qkv, w_proj, b_proj):
    nc = get_nc()
    in_maps = shard_inputs(x, w_qkv, b_qkv, w_proj, b_proj)
    res = run_bass_kernel_spmd(nc, in_maps, core_ids=list(range(NCORES)))
    return gather_output(res.results, b_proj)


# revision 42
# speedup vs baseline: 1.0297x; 1.0297x over previous
"""Multi-head self-attention (B=4, N=2048, C=512, H=8) on 8 trn2 NeuronCores.

Sharding: core = 2*b + g (b = batch, g = head-half). Each core handles one
batch element and 4 heads (2 head-pairs j); host sums the two partial
projections per batch element and adds b_proj.

v4 design (all attention matmuls bf16, fp32 PSUM accumulation):
  0. startup: inputs spread over the three dma-capable queues (sync/
     scalar/gpsimd, ~90 B/ns each; a dma_start kick costs ~0.7us of
     engine time, so kick order = critical path order); 24 dummy
     accumulating matmuls on memset data warm the PE clock (HAM 1.2->2.4
     GHz) while x streams in.
  1. qkv: q^T/k^T per pair j as [128, 2048] bf16, ct-outer in nkq-pairs
     so the first matmuls need only the first x tiles. v packed per
     key-tile as [128, 4, 66] bf16 with a ones column at index 64
     (softmax-denominator trick); v matmuls share the qk PSUM pool.
     Early-exp: scores+exp for (j0,q5=0) run during the qkv phase
     interleaved with the v matmuls, alternating ACT/DVE per key-tile.
  2. attention, software-pipelined: scores for key-tile m+LOOK issue
     ahead of AV matmuls for tile m, so exp (ACT, with DVE-Schraudolph
     tiles per DVE_MS/SPLIT_M balancing the two queues) is never on the
     PE's critical path. PSUM: 3 score tiles (6 banks) + oTa/oTb (2).
     Group-1 scores pre-issue through group-0's early AVs (PRE).
  3. normalize, split: prompt part frees the PSUM banks (den-row copy on
     DVE, body copy on ACT); reciprocal + Pool broadcast + DVE multiply
     are deferred into the next group's stream so they never block exps.
     NB: custom DVE ops (reciprocal) silently corrupt on non-zero base
     partitions on HW (sim does not model this), and ACT APs must be
     32-aligned in partitions - hence the staging copies.
  4. projection: y^T accumulated over the two pairs per q-chunk, bf16
     output (host gathers in fp32), 16 output DMAs spread over the three
     queues so the 2MB drain overlaps the last attention group.
"""

from collections import deque

import numpy as np

import concourse.bacc as bacc
import concourse.bass as bass
import concourse.mybir as mybir
import concourse.tile as tile
from concourse.bass_utils import run_bass_kernel_spmd

B, N, C, H, HD = 4, 2048, 512, 8, 64
HPC, CS = 4, 256  # heads per core, channels per core
SCALE = HD ** -0.5
F32 = mybir.dt.float32
BF16 = mybir.dt.bfloat16
U16 = mybir.dt.uint16
NCORES = 8
MT = N // 128  # 16 key tiles

LOG2E = float(np.log2(np.e))
SCH_A = SCALE * 128.0 * LOG2E   # schraudolph scale (bf16 bits)
SCH_B = 16256.0 - 5.5           # 127<<7 minus minimax correction

# key-tiles m whose exp runs on DVE (Schraudolph); rest use exact ACT exp
DVE_MS = frozenset({1, 3, 5, 7, 11, 13})
SPLIT_M = 15  # this tile's exp is half ACT / half DVE
EARLY_EXP = True
LOOK = 5  # score tiles issued ahead of AV matmuls

_NC = None


def _build(reps=1):
    nc = bacc.Bacc("TRN2", target_bir_lowering=False, debug=False,
                   num_devices=NCORES)
    xT_d = nc.dram_tensor("xT", [C, N], BF16, kind="ExternalInput")
    wqT_d = nc.dram_tensor("wqT", [C, CS], BF16, kind="ExternalInput")
    wkT_d = nc.dram_tensor("wkT", [C, CS], BF16, kind="ExternalInput")
    wvT_d = nc.dram_tensor("wvT", [C, CS], BF16, kind="ExternalInput")
    wpT_d = nc.dram_tensor("wpT", [CS, C], BF16, kind="ExternalInput")
    bq_d = nc.dram_tensor("bq", [128, 2], F32, kind="ExternalInput")
    bk_d = nc.dram_tensor("bk", [128, 2], F32, kind="ExternalInput")
    bv_d = nc.dram_tensor("bv", [1, CS], BF16, kind="ExternalInput")
    ones4_d = nc.dram_tensor("ones4", [128, HPC], BF16, kind="ExternalInput")
    ones_row_d = nc.dram_tensor("ones_row", [1, 128], BF16,
                                kind="ExternalInput")
    yT_d = nc.dram_tensor("yT", [C, N], BF16, kind="ExternalOutput")

    with tile.TileContext(nc) as tc:
      def body():
          with (
              tc.tile_pool(name="const", bufs=1) as const,
              tc.tile_pool(name="big", bufs=1) as big,
              tc.tile_pool(name="pexp", bufs=8) as pexp,
              tc.tile_pool(name="psch", bufs=8) as psch,
              tc.tile_pool(name="pearly", bufs=1) as pearly,
              tc.tile_pool(name="rc", bufs=3) as rcp,
              tc.tile_pool(name="rbc", bufs=3) as rbcp,
              tc.tile_pool(name="osb", bufs=4) as osbp,
              tc.tile_pool(name="ysb", bufs=4) as ysbp,
          ):
              # ---- input DMA ------------------------------------------------
              xt = [big.tile([128, N], BF16, tag=f"x{ct}", name=f"x{ct}")
                    for ct in range(4)]
              wq_t, wk_t, wv_t = [], [], []
              for ct in range(4):
                  for lst, nm in ((wq_t, "wq"), (wk_t, "wk"), (wv_t, "wv")):
                      lst.append(const.tile([128, CS], BF16, tag=f"{nm}{ct}",
                                            name=f"{nm}{ct}"))
              # critical-path order: qk ct-outer needs xt[0]+wk0/wq0 first;
              # spread x over the three dma-capable queues
              nc.sync.dma_start(out=xt[0][:], in_=xT_d[bass.ts(0, 128), :])
              nc.scalar.dma_start(out=xt[1][:], in_=xT_d[bass.ts(1, 128), :])
              nc.gpsimd.dma_start(out=xt[2][:], in_=xT_d[bass.ts(2, 128), :])
              nc.sync.dma_start(out=xt[3][:, 0:1024],
                                in_=xT_d[bass.ts(3, 128), 0:1024])
              nc.scalar.dma_start(out=xt[3][:, 1024:2048],
                                  in_=xT_d[bass.ts(3, 128), 1024:2048])
              nc.gpsimd.dma_start(out=wk_t[0][:],
                                  in_=wkT_d[bass.ts(0, 128), :])
              nc.gpsimd.dma_start(out=wq_t[0][:],
                                  in_=wqT_d[bass.ts(0, 128), :])
              bk_sb = const.tile([128, 2], F32, tag="bk", name="bk")
              nc.gpsimd.dma_start(out=bk_sb[:], in_=bk_d[:])
              bq_sb = const.tile([128, 2], F32, tag="bq", name="bq")
              nc.gpsimd.dma_start(out=bq_sb[:], in_=bq_d[:])
              nc.sync.dma_start(out=wk_t[1][:], in_=wkT_d[bass.ts(1, 128), :])
              nc.sync.dma_start(out=wk_t[3][:], in_=wkT_d[bass.ts(3, 128), :])
              nc.scalar.dma_start(out=wq_t[1][:],
                                  in_=wqT_d[bass.ts(1, 128), :])
              nc.scalar.dma_start(out=wq_t[3][:],
                                  in_=wqT_d[bass.ts(3, 128), :])
              nc.gpsimd.dma_start(out=wk_t[2][:],
                                  in_=wkT_d[bass.ts(2, 128), :])
              nc.gpsimd.dma_start(out=wq_t[2][:],
                                  in_=wqT_d[bass.ts(2, 128), :])
              for ct in range(4):
                  nc.scalar.dma_start(out=wv_t[ct][:],
                                      in_=wvT_d[bass.ts(ct, 128), :])
              bv_sb = const.tile([1, CS], BF16, tag="bv", name="bv")
              nc.gpsimd.dma_start(out=bv_sb[:], in_=bv_d[:])
              ones_row = const.tile([1, 128], BF16, tag="ones_row",
                                    name="ones_row")
              nc.gpsimd.dma_start(out=ones_row[:], in_=ones_row_d[:])
              wp_t = []
              for j in range(2):
                  t = const.tile([128, C], BF16, tag=f"wp{j}", name=f"wp{j}")
                  nc.gpsimd.dma_start(out=t[:], in_=wpT_d[bass.ts(j, 128), :])
                  wp_t.append(t)

              # ---- persistent activations -----------------------------------
              qT = [big.tile([128, N], BF16, tag=f"qT{j}", name=f"qT{j}")
                    for j in range(2)]
              kT = [big.tile([128, N], BF16, tag=f"kT{j}", name=f"kT{j}")
                    for j in range(2)]
              v1m = [big.tile([128, HPC, HD + 2], BF16, tag=f"v1m_{m}",
                              name=f"v1m_{m}") for m in range(MT)]
              for m in range(MT):
                  nc.sync.dma_start(
                      out=v1m[m][:, :, HD:HD + 1],
                      in_=ones4_d[:, :].rearrange("p (h o) -> p h o", o=1),
                  )
              oT_sb = [big.tile([128, N], BF16, tag=f"oT{j}", name=f"oT{j}")
                       for j in range(2)]

              # ---- phase 1: qkv ---------------------------------------------
              pt_early = []
              with (
                  tc.tile_pool(name="qkps", bufs=2, space="PSUM") as qkps,
                  tc.tile_pool(name="eps", bufs=3, space="PSUM") as eps,
              ):
                  def qk_pair(j):
                      for w_t, b_sb, dst in ((wk_t, bk_sb, kT),
                                             (wq_t, bq_sb, qT)):
                          for nk0 in (0, 2):
                              pss = [qkps.tile([128, 512], F32, tag="qk",
                                               name="qk") for _ in range(2)]
                              for ct in range(4):
                                  for i, ps in enumerate(pss):
                                      nc.tensor.matmul(
                                          ps[:],
                                          lhsT=w_t[ct][:, bass.ts(j, 128)],
                                          rhs=xt[ct][:, bass.ts(nk0 + i,
                                                                512)],
                                          start=(ct == 0), stop=(ct == 3),
                                      )
                              for i, ps in enumerate(pss):
                                  nc.vector.tensor_scalar_add(
                                      dst[j][:, bass.ts(nk0 + i, 512)],
                                      ps[:], b_sb[:, j:j + 1])

                  # dummy matmuls on memset data warm the PE clock (HAM)
                  # during the input DMA so qk runs at 2.4 GHz
                  warm = const.tile([128, 512], BF16, tag="warm",
                                    name="warm")
                  nc.vector.memset(warm[:], 0.0)
                  wps = eps.tile([128, 1024], F32, tag="es", name="warmps")
                  NWARM = 24
                  for i in range(NWARM):
                      nc.tensor.matmul(wps[:, 0:512], lhsT=warm[:, 0:128],
                                       rhs=warm[:], start=(i == 0),
                                       stop=(i == NWARM - 1))
                  qk_pair(0)
                  # early-exp for (j0, q5=0) interleaved with the v matmuls
                  for m in range(MT):
                      if EARLY_EXP:
                          esT = eps.tile([128, 1024], F32, tag="es",
                                         name="es")
                          nc.tensor.matmul(
                              esT[:, 0:512],
                              lhsT=kT[0][0:64, bass.ts(m, 128)],
                              rhs=qT[0][0:64, 0:512],
                              start=True, stop=True)
                          nc.tensor.matmul(
                              esT[:, 512:1024],
                              lhsT=kT[0][64:128, bass.ts(m, 128)],
                              rhs=qT[0][64:128, 0:512],
                              start=True, stop=True)
                          if m % 2 == 0:
                              ept = pearly.tile([128, 1024], BF16,
                                                tag=f"ep{m}", name=f"ep{m}")
                              nc.scalar.activation(
                                  out=ept[:], in_=esT[:],
                                  func=mybir.ActivationFunctionType.Exp,
                                  scale=SCALE)
                              pt_early.append(
                                  (ept[:, 0:512], ept[:, 512:1024]))
                          else:
                              epu = pearly.tile([128, 1024], U16,
                                                tag=f"ep{m}", name=f"ep{m}")
                              nc.vector.tensor_scalar(
                                  out=epu[:], in0=esT[:],
                                  scalar1=SCH_A, scalar2=SCH_B,
                                  op0=mybir.AluOpType.mult,
                                  op1=mybir.AluOpType.add)
                              pt_early.append(
                                  (epu[:, 0:512].bitcast(BF16),
                                   epu[:, 512:1024].bitcast(BF16)))
                      vp = qkps.tile([128, 512], F32, tag="qk", name="vps")
                      for ct in range(4):
                          nc.tensor.matmul(
                              vp[:, 0:CS],
                              lhsT=xt[ct][:, bass.ts(m, 128)],
                              rhs=wv_t[ct][:],
                              start=(ct == 0), stop=False,
                          )
                      nc.tensor.matmul(vp[:, 0:CS], lhsT=ones_row[:],
                                       rhs=bv_sb[:],
                                       start=False, stop=True)
                      if m % 2 == 0:
                          nc.scalar.copy(v1m[m][:, :, 0:HD], vp[:, 0:CS])
                      else:
                          nc.vector.tensor_copy(v1m[m][:, :, 0:HD],
                                                vp[:, 0:CS])
                  qk_pair(1)

              # ---- phase 2: attention (software pipeline) -------------------
              groups = [(j, q5) for j in range(2) for q5 in range(4)]
              with (
                  tc.tile_pool(name="stps", bufs=3, space="PSUM") as stps,
                  tc.tile_pool(name="otps", bufs=1, space="PSUM") as otps,
              ):
                  def issue_scores(gi, m):
                      j, q5 = groups[gi]
                      if EARLY_EXP and gi == 0:
                          return pt_early[m]
                      sT = stps.tile([128, 1024], F32, tag="s", name="s")
                      nc.tensor.matmul(
                          sT[:, 0:512],
                          lhsT=kT[j][0:64, bass.ts(m, 128)],
                          rhs=qT[j][0:64, bass.ts(q5, 512)],
                          start=True, stop=True)
                      nc.tensor.matmul(
                          sT[:, 512:1024],
                          lhsT=kT[j][64:128, bass.ts(m, 128)],
                          rhs=qT[j][64:128, bass.ts(q5, 512)],
                          start=True, stop=True)
                      if m == SPLIT_M:
                          # half-split tile to fine-balance ACT/DVE load
                          pa = pexp.tile([128, 512], BF16, tag="pes",
                                         name="pes")
                          nc.scalar.activation(
                              out=pa[:], in_=sT[:, 0:512],
                              func=mybir.ActivationFunctionType.Exp,
                              scale=SCALE)
                          pb = psch.tile([128, 512], U16, tag="pss",
                                         name="pss")
                          nc.vector.tensor_scalar(
                              out=pb[:], in0=sT[:, 512:1024],
                              scalar1=SCH_A, scalar2=SCH_B,
                              op0=mybir.AluOpType.mult,
                              op1=mybir.AluOpType.add)
                          return (pa[:], pb[:].bitcast(BF16))
                      if m in DVE_MS:
                          pt = psch.tile([128, 1024], U16, tag="ps",
                                         name="ps")
                          nc.vector.tensor_scalar(
                              out=pt[:], in0=sT[:],
                              scalar1=SCH_A, scalar2=SCH_B,
                              op0=mybir.AluOpType.mult,
                              op1=mybir.AluOpType.add)
                          return (pt[:, 0:512].bitcast(BF16),
                                  pt[:, 512:1024].bitcast(BF16))
                      pt = pexp.tile([128, 1024], BF16, tag="pe", name="pe")
                      nc.scalar.activation(
                          out=pt[:], in_=sT[:],
                          func=mybir.ActivationFunctionType.Exp,
                          scale=SCALE)
                      return (pt[:, 0:512], pt[:, 512:1024])

                  def issue_av(gi, m, rhs_ab, oT_pair):
                      j, q5 = groups[gi]
                      for hh, (oT, rhs) in enumerate(zip(oT_pair, rhs_ab)):
                          nc.tensor.matmul(
                              oT[:],
                              lhsT=v1m[m][:, 2 * j + hh, 0:HD + 1],
                              rhs=rhs,
                              start=(m == 0), stop=(m == MT - 1))

                  deferred = []

                  def finalize_a(gi, oT_pair):
                      # prompt part: free the PSUM banks; denominator sits
                      # at partition 0 so recip reads PSUM directly
                      j, q5 = groups[gi]
                      for hh, oT in enumerate(oT_pair):
                          den = rcp.tile([1, 512], F32, tag="den",
                                         name="den")
                          nc.vector.tensor_copy(den[:], oT[HD:HD + 1, :])
                          osb = osbp.tile([HD, 512], F32, tag="osb",
                                          name="osb")
                          nc.scalar.copy(osb[:], oT[0:HD, :])
                          deferred.append((j, q5, hh, den, osb))

                  def finalize_b():
                      while deferred:
                          j, q5, hh, den, osb = deferred.pop(0)
                          rc = rcp.tile([1, 512], F32, tag="rc", name="rc")
                          nc.vector.reciprocal_approx_fast(
                              out=rc[:], in_=den[:])
                          bc = rbcp.tile([HD, 512], F32, tag="bc", name="bc")
                          nc.gpsimd.partition_broadcast(bc[:], rc[:])
                          nc.vector.tensor_mul(
                              oT_sb[j][bass.ts(hh, 64), bass.ts(q5, 512)],
                              osb[:], bc[:])

                  def alloc_pair():
                      return (
                          otps.tile([HD + 1, 512], F32, tag="oa", name="oa"),
                          otps.tile([HD + 1, 512], F32, tag="ob", name="ob"),
                      )

                  if EARLY_EXP:
                      # group 0's AVs read precomputed SBUF tiles, so
                      # group 1's scores+exps interleave through them.
                      # PRE bounded by exp-output capacity (6 ACT + 6 DVE).
                      PRE = 12
                      oT0 = alloc_pair()
                      q1 = deque()
                      for m in range(MT):
                          if 0 <= m - 2 < PRE:
                              q1.append((m - 2, issue_scores(1, m - 2)))
                          issue_av(0, m, pt_early[m], oT0)
                      finalize_a(0, oT0)
                      oT1 = alloc_pair()
                      for m in range(PRE, MT):
                          q1.append((m, issue_scores(1, m)))
                          pm, rhs = q1.popleft()
                          issue_av(1, pm, rhs, oT1)
                          if pm == 4:
                              finalize_b()
                      while q1:
                          pm, rhs = q1.popleft()
                          issue_av(1, pm, rhs, oT1)
                      finalize_a(1, oT1)
                      start_gi = 2
                  else:
                      start_gi = 0

                  for gi in range(start_gi, len(groups)):
                      oT_pair = alloc_pair()
                      q = deque()
                      for m in range(MT):
                          q.append((m, issue_scores(gi, m)))
                          if len(q) > LOOK:
                              pm, rhs = q.popleft()
                              issue_av(gi, pm, rhs, oT_pair)
                              if pm == 4:
                                  finalize_b()
                      while q:
                          pm, rhs = q.popleft()
                          issue_av(gi, pm, rhs, oT_pair)
                      finalize_a(gi, oT_pair)
                  finalize_b()
              # ---- phase 3: projection --------------------------------------
              with tc.tile_pool(name="yps", bufs=4, space="PSUM") as yps:
                  yqs = [nc.sync, nc.gpsimd, nc.scalar]
                  for tch in range(4):
                      for jj in range(4):
                          yp = yps.tile([128, 512], F32, tag="yp",
                                        name="yp")
                          for j in range(2):
                              nc.tensor.matmul(
                                  yp[:],
                                  lhsT=wp_t[j][:, bass.ts(jj, 128)],
                                  rhs=oT_sb[j][:, bass.ts(tch, 512)],
                                  start=(j == 0), stop=(j == 1))
                          ys = ysbp.tile([128, 512], BF16, tag="ys",
                                         name="ys")
                          if jj % 2 == 0:
                              nc.scalar.copy(ys[:], yp[:])
                          else:
                              nc.vector.tensor_copy(ys[:], yp[:])
                          yqs[(4 * tch + jj) % 3].dma_start(
                              out=yT_d[bass.ts(jj, 128), bass.ts(tch, 512)],
                              in_=ys[:])

      if reps > 1:
          with tc.For_i(0, reps, 1):
              body()
      else:
          body()

    nc.compile()
    return nc


def get_nc():
    global _NC
    if _NC is None:
        _NC = _build()
    return _NC


def build_timing_nc(reps):
    return _build(reps=reps)


def shard_inputs(x, w_qkv, b_qkv, w_proj, b_proj):
    import ml_dtypes

    bf16 = ml_dtypes.bfloat16
    x = np.asarray(x, dtype=np.float32)
    w_qkv = np.asarray(w_qkv, dtype=np.float32)
    b_qkv = np.asarray(b_qkv, dtype=np.float32)
    w_proj = np.asarray(w_proj, dtype=np.float32)
    ones4 = np.ones((128, HPC), bf16)
    ones_row = np.ones((1, 128), bf16)
    in_maps = []
    for core in range(NCORES):
        b, g = core // 2, core % 2
        sl = slice(g * CS, (g + 1) * CS)
        in_maps.append({
            "xT": np.ascontiguousarray(x[b].T).astype(bf16),
            "wqT": np.ascontiguousarray(w_qkv[sl, :].T).astype(bf16),
            "wkT": np.ascontiguousarray(w_qkv[C:][sl, :].T).astype(bf16),
            "wvT": np.ascontiguousarray(w_qkv[2 * C:][sl, :].T).astype(bf16),
            "wpT": np.ascontiguousarray(w_proj[:, sl].T).astype(bf16),
            "bq": np.ascontiguousarray(b_qkv[sl].reshape(2, 128).T),
            "bk": np.ascontiguousarray(b_qkv[C:][sl].reshape(2, 128).T),
            "bv": np.ascontiguousarray(
                b_qkv[2 * C:][sl].reshape(1, CS)).astype(bf16),
            "ones4": ones4,
            "ones_row": ones_row,
        })
    return in_maps


def gather_output(results, b_proj):
    b_proj = np.asarray(b_proj, dtype=np.float32)
    out = np.empty((B, N, C), np.float32)
    for b in range(B):
        yT = (results[2 * b]["yT"].astype(np.float32)
              + results[2 * b + 1]["yT"].astype(np.float32))
        out[b] = yT.T + b_proj[None, :]
    return out


def kernel(x, w_qkv, b_qkv, w_proj, b_proj):
    nc = get_nc()
    in_maps = shard_inputs(x, w_qkv, b_qkv, w_proj, b_proj)
    res = run_bass_kernel_spmd(nc, in_maps, core_ids=list(range(NCORES)))
    return gather_output(res.results, b_proj)


# revision 43
# speedup vs baseline: 1.0358x; 1.0060x over previous
"""Multi-head self-attention (B=4, N=2048, C=512, H=8) on 8 trn2 NeuronCores.

Sharding: core = 2*b + g (b = batch, g = head-half). Each core handles one
batch element and 4 heads (2 head-pairs j); host sums the two partial
projections per batch element and adds b_proj.

v4 design (all attention matmuls bf16, fp32 PSUM accumulation):
  0. startup: inputs spread over the three dma-capable queues (sync/
     scalar/gpsimd, ~90 B/ns each; a dma_start kick costs ~0.7us of
     engine time, so kick order = critical path order); 24 dummy
     accumulating matmuls on memset data warm the PE clock (HAM 1.2->2.4
     GHz) while x streams in.
  1. qkv: q^T/k^T per pair j as [128, 2048] bf16, ct-outer in nkq-pairs
     so the first matmuls need only the first x tiles. v packed per
     key-tile as [128, 4, 66] bf16 with a ones column at index 64
     (softmax-denominator trick); v matmuls share the qk PSUM pool.
     Early-exp: scores+exp for (j0,q5=0) run during the qkv phase
     interleaved with the v matmuls, alternating ACT/DVE per key-tile.
  2. attention, software-pipelined: scores for key-tile m+LOOK issue
     ahead of AV matmuls for tile m, so exp (ACT, with DVE-Schraudolph
     tiles per DVE_MS/SPLIT_M balancing the two queues) is never on the
     PE's critical path. PSUM: 3 score tiles (6 banks) + oTa/oTb (2).
     Group-1 scores pre-issue through group-0's early AVs (PRE).
  3. normalize, split: prompt part frees the PSUM banks (den-row copy on
     DVE, body copy on ACT); reciprocal + Pool broadcast + DVE multiply
     are deferred into the next group's stream so they never block exps.
     NB: custom DVE ops (reciprocal) silently corrupt on non-zero base
     partitions on HW (sim does not model this), and ACT APs must be
     32-aligned in partitions - hence the staging copies.
  4. projection: y^T accumulated over the two pairs per q-chunk, bf16
     output (host gathers in fp32), 16 output DMAs spread over the three
     queues so the 2MB drain overlaps the last attention group.
"""

from collections import deque

import numpy as np

import concourse.bacc as bacc
import concourse.bass as bass
import concourse.mybir as mybir
import concourse.tile as tile
from concourse.bass_utils import run_bass_kernel_spmd

B, N, C, H, HD = 4, 2048, 512, 8, 64
HPC, CS = 4, 256  # heads per core, channels per core
SCALE = HD ** -0.5
F32 = mybir.dt.float32
BF16 = mybir.dt.bfloat16
U16 = mybir.dt.uint16
NCORES = 8
MT = N // 128  # 16 key tiles

LOG2E = float(np.log2(np.e))
SCH_A = SCALE * 128.0 * LOG2E   # schraudolph scale (bf16 bits)
SCH_B = 16256.0 - 5.5           # 127<<7 minus minimax correction

# key-tiles m whose exp runs on DVE (Schraudolph); rest use exact ACT exp
DVE_MS = frozenset({1, 3, 5, 7, 11, 13})
SPLIT_M = 15  # this tile's exp is half ACT / half DVE
EARLY_EXP = True
LOOK = 5  # score tiles issued ahead of AV matmuls

_NC = None


def _build(reps=1):
    nc = bacc.Bacc("TRN2", target_bir_lowering=False, debug=False,
                   num_devices=NCORES)
    xT_d = nc.dram_tensor("xT", [C, N], BF16, kind="ExternalInput")
    wqT_d = nc.dram_tensor("wqT", [C, CS], BF16, kind="ExternalInput")
    wkT_d = nc.dram_tensor("wkT", [C, CS], BF16, kind="ExternalInput")
    wvT_d = nc.dram_tensor("wvT", [C, CS], BF16, kind="ExternalInput")
    wpT_d = nc.dram_tensor("wpT", [CS, C], BF16, kind="ExternalInput")
    bq_d = nc.dram_tensor("bq", [128, 2], F32, kind="ExternalInput")
    bk_d = nc.dram_tensor("bk", [128, 2], F32, kind="ExternalInput")
    ones4_d = nc.dram_tensor("ones4", [128, HPC], BF16, kind="ExternalInput")
    yT_d = nc.dram_tensor("yT", [C, N], BF16, kind="ExternalOutput")

    with tile.TileContext(nc) as tc:
      def body():
          with (
              tc.tile_pool(name="const", bufs=1) as const,
              tc.tile_pool(name="big", bufs=1) as big,
              tc.tile_pool(name="pexp", bufs=8) as pexp,
              tc.tile_pool(name="psch", bufs=8) as psch,
              tc.tile_pool(name="pearly", bufs=1) as pearly,
              tc.tile_pool(name="rc", bufs=3) as rcp,
              tc.tile_pool(name="rbc", bufs=3) as rbcp,
              tc.tile_pool(name="osb", bufs=4) as osbp,
              tc.tile_pool(name="ysb", bufs=4) as ysbp,
          ):
              # ---- input DMA ------------------------------------------------
              xt = [big.tile([128, N], BF16, tag=f"x{ct}", name=f"x{ct}")
                    for ct in range(4)]
              wq_t, wk_t, wv_t = [], [], []
              for ct in range(4):
                  for lst, nm in ((wq_t, "wq"), (wk_t, "wk"), (wv_t, "wv")):
                      lst.append(const.tile([128, CS], BF16, tag=f"{nm}{ct}",
                                            name=f"{nm}{ct}"))
              # critical-path order: qk ct-outer needs xt[0]+wk0/wq0 first;
              # spread x over the three dma-capable queues
              nc.sync.dma_start(out=xt[0][:], in_=xT_d[bass.ts(0, 128), :])
              nc.scalar.dma_start(out=xt[1][:], in_=xT_d[bass.ts(1, 128), :])
              nc.gpsimd.dma_start(out=xt[2][:], in_=xT_d[bass.ts(2, 128), :])
              nc.sync.dma_start(out=xt[3][:, 0:1024],
                                in_=xT_d[bass.ts(3, 128), 0:1024])
              nc.scalar.dma_start(out=xt[3][:, 1024:2048],
                                  in_=xT_d[bass.ts(3, 128), 1024:2048])
              nc.gpsimd.dma_start(out=wk_t[0][:],
                                  in_=wkT_d[bass.ts(0, 128), :])
              nc.gpsimd.dma_start(out=wq_t[0][:],
                                  in_=wqT_d[bass.ts(0, 128), :])
              bk_sb = const.tile([128, 2], F32, tag="bk", name="bk")
              nc.gpsimd.dma_start(out=bk_sb[:], in_=bk_d[:])
              bq_sb = const.tile([128, 2], F32, tag="bq", name="bq")
              nc.gpsimd.dma_start(out=bq_sb[:], in_=bq_d[:])
              nc.sync.dma_start(out=wk_t[1][:], in_=wkT_d[bass.ts(1, 128), :])
              nc.sync.dma_start(out=wk_t[3][:], in_=wkT_d[bass.ts(3, 128), :])
              nc.scalar.dma_start(out=wq_t[1][:],
                                  in_=wqT_d[bass.ts(1, 128), :])
              nc.scalar.dma_start(out=wq_t[3][:],
                                  in_=wqT_d[bass.ts(3, 128), :])
              nc.gpsimd.dma_start(out=wk_t[2][:],
                                  in_=wkT_d[bass.ts(2, 128), :])
              nc.gpsimd.dma_start(out=wq_t[2][:],
                                  in_=wqT_d[bass.ts(2, 128), :])
              for ct in range(4):
                  nc.scalar.dma_start(out=wv_t[ct][:],
                                      in_=wvT_d[bass.ts(ct, 128), :])
              wp_t = []
              for j in range(2):
                  t = const.tile([128, C], BF16, tag=f"wp{j}", name=f"wp{j}")
                  nc.gpsimd.dma_start(out=t[:], in_=wpT_d[bass.ts(j, 128), :])
                  wp_t.append(t)

              # ---- persistent activations -----------------------------------
              qT = [big.tile([128, N], BF16, tag=f"qT{j}", name=f"qT{j}")
                    for j in range(2)]
              kT = [big.tile([128, N], BF16, tag=f"kT{j}", name=f"kT{j}")
                    for j in range(2)]
              v1m = [big.tile([128, HPC, HD + 2], BF16, tag=f"v1m_{m}",
                              name=f"v1m_{m}") for m in range(MT)]
              for m in range(MT):
                  nc.sync.dma_start(
                      out=v1m[m][:, :, HD:HD + 1],
                      in_=ones4_d[:, :].rearrange("p (h o) -> p h o", o=1),
                  )
              oT_sb = [big.tile([128, N], BF16, tag=f"oT{j}", name=f"oT{j}")
                       for j in range(2)]

              # ---- phase 1: qkv ---------------------------------------------
              pt_early = []
              with (
                  tc.tile_pool(name="qkps", bufs=2, space="PSUM") as qkps,
                  tc.tile_pool(name="eps", bufs=3, space="PSUM") as eps,
              ):
                  def qk_pair(j):
                      for w_t, b_sb, dst in ((wk_t, bk_sb, kT),
                                             (wq_t, bq_sb, qT)):
                          for nk0 in (0, 2):
                              pss = [qkps.tile([128, 512], F32, tag="qk",
                                               name="qk") for _ in range(2)]
                              for ct in range(4):
                                  for i, ps in enumerate(pss):
                                      nc.tensor.matmul(
                                          ps[:],
                                          lhsT=w_t[ct][:, bass.ts(j, 128)],
                                          rhs=xt[ct][:, bass.ts(nk0 + i,
                                                                512)],
                                          start=(ct == 0), stop=(ct == 3),
                                      )
                              for i, ps in enumerate(pss):
                                  nc.vector.tensor_scalar_add(
                                      dst[j][:, bass.ts(nk0 + i, 512)],
                                      ps[:], b_sb[:, j:j + 1])

                  # dummy matmuls on memset data warm the PE clock (HAM)
                  # during the input DMA so qk runs at 2.4 GHz
                  warm = const.tile([128, 512], BF16, tag="warm",
                                    name="warm")
                  nc.vector.memset(warm[:], 0.0)
                  wps = eps.tile([128, 1024], F32, tag="es", name="warmps")
                  NWARM = 24
                  for i in range(NWARM):
                      nc.tensor.matmul(wps[:, 0:512], lhsT=warm[:, 0:128],
                                       rhs=warm[:], start=(i == 0),
                                       stop=(i == NWARM - 1))
                  qk_pair(0)
                  # early-exp for (j0, q5=0) interleaved with the v matmuls
                  for m in range(MT):
                      if EARLY_EXP:
                          esT = eps.tile([128, 1024], F32, tag="es",
                                         name="es")
                          nc.tensor.matmul(
                              esT[:, 0:512],
                              lhsT=kT[0][0:64, bass.ts(m, 128)],
                              rhs=qT[0][0:64, 0:512],
                              start=True, stop=True)
                          nc.tensor.matmul(
                              esT[:, 512:1024],
                              lhsT=kT[0][64:128, bass.ts(m, 128)],
                              rhs=qT[0][64:128, 0:512],
                              start=True, stop=True)
                          if m % 2 == 0:
                              ept = pearly.tile([128, 1024], BF16,
                                                tag=f"ep{m}", name=f"ep{m}")
                              nc.scalar.activation(
                                  out=ept[:], in_=esT[:],
                                  func=mybir.ActivationFunctionType.Exp,
                                  scale=SCALE)
                              pt_early.append(
                                  (ept[:, 0:512], ept[:, 512:1024]))
                          else:
                              epu = pearly.tile([128, 1024], U16,
                                                tag=f"ep{m}", name=f"ep{m}")
                              nc.vector.tensor_scalar(
                                  out=epu[:], in0=esT[:],
                                  scalar1=SCH_A, scalar2=SCH_B,
                                  op0=mybir.AluOpType.mult,
                                  op1=mybir.AluOpType.add)
                              pt_early.append(
                                  (epu[:, 0:512].bitcast(BF16),
                                   epu[:, 512:1024].bitcast(BF16)))
                      vp = qkps.tile([128, 512], F32, tag="qk", name="vps")
                      for ct in range(4):
                          nc.tensor.matmul(
                              vp[:, 0:CS],
                              lhsT=xt[ct][:, bass.ts(m, 128)],
                              rhs=wv_t[ct][:],
                              start=(ct == 0), stop=(ct == 3),
                          )
                      if m % 2 == 0:
                          nc.scalar.copy(v1m[m][:, :, 0:HD], vp[:, 0:CS])
                      else:
                          nc.vector.tensor_copy(v1m[m][:, :, 0:HD],
                                                vp[:, 0:CS])
                  qk_pair(1)

              # ---- phase 2: attention (software pipeline) -------------------
              groups = [(j, q5) for j in range(2) for q5 in range(4)]
              with (
                  tc.tile_pool(name="stps", bufs=3, space="PSUM") as stps,
                  tc.tile_pool(name="otps", bufs=1, space="PSUM") as otps,
              ):
                  def issue_scores(gi, m):
                      j, q5 = groups[gi]
                      if EARLY_EXP and gi == 0:
                          return pt_early[m]
                      sT = stps.tile([128, 1024], F32, tag="s", name="s")
                      nc.tensor.matmul(
                          sT[:, 0:512],
                          lhsT=kT[j][0:64, bass.ts(m, 128)],
                          rhs=qT[j][0:64, bass.ts(q5, 512)],
                          start=True, stop=True)
                      nc.tensor.matmul(
                          sT[:, 512:1024],
                          lhsT=kT[j][64:128, bass.ts(m, 128)],
                          rhs=qT[j][64:128, bass.ts(q5, 512)],
                          start=True, stop=True)
                      if m == SPLIT_M:
                          # half-split tile to fine-balance ACT/DVE load
                          pa = pexp.tile([128, 512], BF16, tag="pes",
                                         name="pes")
                          nc.scalar.activation(
                              out=pa[:], in_=sT[:, 0:512],
                              func=mybir.ActivationFunctionType.Exp,
                              scale=SCALE)
                          pb = psch.tile([128, 512], U16, tag="pss",
                                         name="pss")
                          nc.vector.tensor_scalar(
                              out=pb[:], in0=sT[:, 512:1024],
                              scalar1=SCH_A, scalar2=SCH_B,
                              op0=mybir.AluOpType.mult,
                              op1=mybir.AluOpType.add)
                          return (pa[:], pb[:].bitcast(BF16))
                      if m in DVE_MS:
                          pt = psch.tile([128, 1024], U16, tag="ps",
                                         name="ps")
                          nc.vector.tensor_scalar(
                              out=pt[:], in0=sT[:],
                              scalar1=SCH_A, scalar2=SCH_B,
                              op0=mybir.AluOpType.mult,
                              op1=mybir.AluOpType.add)
                          return (pt[:, 0:512].bitcast(BF16),
                                  pt[:, 512:1024].bitcast(BF16))
                      pt = pexp.tile([128, 1024], BF16, tag="pe", name="pe")
                      nc.scalar.activation(
                          out=pt[:], in_=sT[:],
                          func=mybir.ActivationFunctionType.Exp,
                          scale=SCALE)
                      return (pt[:, 0:512], pt[:, 512:1024])

                  def issue_av(gi, m, rhs_ab, oT_pair):
                      j, q5 = groups[gi]
                      for hh, (oT, rhs) in enumerate(zip(oT_pair, rhs_ab)):
                          nc.tensor.matmul(
                              oT[:],
                              lhsT=v1m[m][:, 2 * j + hh, 0:HD + 1],
                              rhs=rhs,
                              start=(m == 0), stop=(m == MT - 1))

                  deferred = []

                  def finalize_a(gi, oT_pair):
                      # prompt part: free the PSUM banks; denominator sits
                      # at partition 0 so recip reads PSUM directly
                      j, q5 = groups[gi]
                      for hh, oT in enumerate(oT_pair):
                          den = rcp.tile([1, 512], F32, tag="den",
                                         name="den")
                          nc.vector.tensor_copy(den[:], oT[HD:HD + 1, :])
                          osb = osbp.tile([HD, 512], F32, tag="osb",
                                          name="osb")
                          nc.scalar.copy(osb[:], oT[0:HD, :])
                          deferred.append((j, q5, hh, den, osb))

                  def finalize_b():
                      while deferred:
                          j, q5, hh, den, osb = deferred.pop(0)
                          rc = rcp.tile([1, 512], F32, tag="rc", name="rc")
                          nc.vector.reciprocal_approx_fast(
                              out=rc[:], in_=den[:])
                          bc = rbcp.tile([HD, 512], F32, tag="bc", name="bc")
                          nc.gpsimd.partition_broadcast(bc[:], rc[:])
                          nc.vector.tensor_mul(
                              oT_sb[j][bass.ts(hh, 64), bass.ts(q5, 512)],
                              osb[:], bc[:])

                  def alloc_pair():
                      return (
                          otps.tile([HD + 1, 512], F32, tag="oa", name="oa"),
                          otps.tile([HD + 1, 512], F32, tag="ob", name="ob"),
                      )

                  if EARLY_EXP:
                      # group 0's AVs read precomputed SBUF tiles, so
                      # group 1's scores+exps interleave through them.
                      # PRE bounded by exp-output capacity (6 ACT + 6 DVE).
                      PRE = 12
                      oT0 = alloc_pair()
                      q1 = deque()
                      for m in range(MT):
                          if 0 <= m - 2 < PRE:
                              q1.append((m - 2, issue_scores(1, m - 2)))
                          issue_av(0, m, pt_early[m], oT0)
                      finalize_a(0, oT0)
                      oT1 = alloc_pair()
                      for m in range(PRE, MT):
                          q1.append((m, issue_scores(1, m)))
                          pm, rhs = q1.popleft()
                          issue_av(1, pm, rhs, oT1)
                          if pm == 4:
                              finalize_b()
                      while q1:
                          pm, rhs = q1.popleft()
                          issue_av(1, pm, rhs, oT1)
                      finalize_a(1, oT1)
                      start_gi = 2
                  else:
                      start_gi = 0

                  for gi in range(start_gi, len(groups)):
                      oT_pair = alloc_pair()
                      q = deque()
                      for m in range(MT):
                          q.append((m, issue_scores(gi, m)))
                          if len(q) > LOOK:
                              pm, rhs = q.popleft()
                              issue_av(gi, pm, rhs, oT_pair)
                              if pm == 4:
                                  finalize_b()
                      while q:
                          pm, rhs = q.popleft()
                          issue_av(gi, pm, rhs, oT_pair)
                      finalize_a(gi, oT_pair)
                  finalize_b()
              # ---- phase 3: projection --------------------------------------
              with tc.tile_pool(name="yps", bufs=4, space="PSUM") as yps:
                  yqs = [nc.sync, nc.gpsimd, nc.scalar]
                  for tch in range(4):
                      for jj in range(4):
                          yp = yps.tile([128, 512], F32, tag="yp",
                                        name="yp")
                          for j in range(2):
                              nc.tensor.matmul(
                                  yp[:],
                                  lhsT=wp_t[j][:, bass.ts(jj, 128)],
                                  rhs=oT_sb[j][:, bass.ts(tch, 512)],
                                  start=(j == 0), stop=(j == 1))
                          ys = ysbp.tile([128, 512], BF16, tag="ys",
                                         name="ys")
                          if jj % 2 == 0:
                              nc.scalar.copy(ys[:], yp[:])
                          else:
                              nc.vector.tensor_copy(ys[:], yp[:])
                          yqs[(4 * tch + jj) % 3].dma_start(
                              out=yT_d[bass.ts(jj, 128), bass.ts(tch, 512)],
                              in_=ys[:])

      if reps > 1:
          with tc.For_i(0, reps, 1):
              body()
      else:
          body()

    nc.compile()
    return nc


def get_nc():
    global _NC
    if _NC is None:
        _NC = _build()
    return _NC


def build_timing_nc(reps):
    return _build(reps=reps)


def shard_inputs(x, w_qkv, b_qkv, w_proj, b_proj):
    import ml_dtypes

    bf16 = ml_dtypes.bfloat16
    x = np.asarray(x, dtype=np.float32)
    w_qkv = np.asarray(w_qkv, dtype=np.float32)
    b_qkv = np.asarray(b_qkv, dtype=np.float32)
    w_proj = np.asarray(w_proj, dtype=np.float32)
    ones4 = np.ones((128, HPC), bf16)
    in_maps = []
    for core in range(NCORES):
        b, g = core // 2, core % 2
        sl = slice(g * CS, (g + 1) * CS)
        in_maps.append({
            "xT": np.ascontiguousarray(x[b].T).astype(bf16),
            "wqT": np.ascontiguousarray(w_qkv[sl, :].T).astype(bf16),
            "wkT": np.ascontiguousarray(w_qkv[C:][sl, :].T).astype(bf16),
            "wvT": np.ascontiguousarray(w_qkv[2 * C:][sl, :].T).astype(bf16),
            "wpT": np.ascontiguousarray(w_proj[:, sl].T).astype(bf16),
            "bq": np.ascontiguousarray(b_qkv[sl].reshape(2, 128).T),
            "bk": np.ascontiguousarray(b_qkv[C:][sl].reshape(2, 128).T),
            "ones4": ones4,
        })
    return in_maps


def gather_output(results, b_proj, w_proj, b_qkv):
    # softmax rows sum to 1, so the v-bias adds b_v to every token's o;
    # through the linear proj that is the constant vector w_proj @ b_v
    b_proj = np.asarray(b_proj, dtype=np.float32)
    bias = b_proj + np.asarray(w_proj, np.float32) @ np.asarray(
        b_qkv, np.float32)[2 * C:]
    out = np.empty((B, N, C), np.float32)
    for b in range(B):
        yT = (results[2 * b]["yT"].astype(np.float32)
              + results[2 * b + 1]["yT"].astype(np.float32))
        out[b] = yT.T + bias[None, :]
    return out


def kernel(x, w_qkv, b_qkv, w_proj, b_proj):
    nc = get_nc()
    in_maps = shard_inputs(x, w_qkv, b_qkv, w_proj, b_proj)
    res = run_bass_kernel_spmd(nc, in_maps, core_ids=list(range(NCORES)))
    return gather_output(res.results, b_proj, w_proj, b_qkv)
